# revision 26
# baseline (speedup 1.0000x reference)
"""MCR2 loss kernel for 8 Trainium2 NeuronCores.

Strategy: the host sorts rows by class label (Gram matrices are invariant
to row order), splits each class evenly across the 8 cores, pads each
per-core class block to a multiple of 512 rows (4 tiles of 128), and casts
to fp8 e4m3 (products accumulate exactly in fp32 PSUM; the quantization
error largely cancels between loss_R and loss_Rc, measured 3.1e-3 vs the
f32 reference whose own rounding error is 3.6e-3).  Each core streams its
2.4 MB shard once and accumulates plain per-class Grams on the tensor
engine: for every "quad" of 4 sample tiles, one [128,128] x [128,128]
matmul (lhsT == rhs == the quad) yields the 4 per-tile 32x32 Grams on the
block diagonal of a [128,128] PSUM region; off-diagonal blocks are free
waste.  No masks, no labels, no vector-engine work on the device.

Perf structure (measured 23.9-25 us on a quiet device vs 224.5 us
baseline; the bass framework preamble/epilogue alone measures ~12.8 us
with a trivial kernel, and the PE rhs-stream floor for this dataflow is
150 quads x 56 ns = 8.4 us, which fully hides the 6.7 us fp8 HBM stream):
- each class's input DMA is split in half across the two HWDGE rings
  (sync/scalar) so SDMA packet round-robin cannot smear completion sems
  past the PE's need time; class 0's head is 16 tiles so the PE starts
  at the first HBM round-trip;
- 5 dummy N=512 matmuls on a zeroed tile pre-warm the PE HAM clock gate
  during the first DMA's ~2.7 us latency, so real matmuls run at 2.4 GHz
  (56 ns/quad) without delaying real work when the gate starts warm;
- accumulators are split into per-bank PSUM tiles (classes 0-3 / 4-7 / 8
  / 9-bulk / 9-tail; Tile tracks deps per-tile, so a shared accumulator
  would false-WAR serialize group copies against later matmuls); each
  finished region is copied and DMA'd out (rings alternated) under later
  compute, and class 9's last 4 quads are peeled into their own region so
  only that short copy+DMA chain (~2.6 us, HBM write receipt bound) trails
  the final matmul.

The host sums the diagonal blocks across quads/cores in float64 and
evaluates the 32x32 logdets.
"""

import sys

sys.path.insert(0, "/opt/trn_rl_repo")

import ml_dtypes
import numpy as np

import concourse.bacc as bacc
import concourse.mybir as mybir
import concourse.tile as tile
from concourse.bass_utils import run_bass_kernel_spmd

N, D, C = 600000, 32, 10
EPS = 0.5
NCORES = 8

_cache = {}


def _build_program(tj):
    """tj: exact per-class tile counts (same on all cores)."""
    TILES = sum(tj)
    ROWS = TILES * 128
    nq9 = tj[C - 1] // 4 + (1 if tj[C - 1] % 4 else 0)
    split9 = nq9 >= 6  # peel the last 4 MMs of class 9 into their own region
    NREG = C + 1 if split9 else C
    MW = NREG * 128  # one [128,128] f32 region per class (+1 for the peel)

    nc = bacc.Bacc(None)
    fp8 = mybir.dt.float8e4
    f32 = mybir.dt.float32
    z_dram = nc.dram_tensor("Z", [ROWS, D], fp8, kind="ExternalInput")
    out_dram = nc.dram_tensor("grams", [128, MW], f32, kind="ExternalOutput")

    # accumulator groups: classes per PSUM tile (each tile pads to its own
    # bank, so copies of a finished group never serialize against matmuls
    # still accumulating into another group)
    groups = [(0, 4), (4, 8), (8, 9)]

    with tile.TileContext(nc) as tc:
        with (
            tc.tile_pool(name="zin", bufs=1) as zin_pool,
            tc.tile_pool(name="warm", bufs=1) as warm_pool,
            tc.tile_pool(name="outp", bufs=1) as out_pool,
            tc.tile_pool(name="psum", bufs=1, space="PSUM") as psum_pool,
            tc.tile_pool(name="psumw", bufs=1, space="PSUM") as psumw_pool,
        ):
            accs = [
                psum_pool.tile(
                    [128, (hi - lo) * 128], f32, tag=f"acc{g}", name=f"acc{g}"
                )
                for g, (lo, hi) in enumerate(groups)
            ]
            acc9a = psum_pool.tile([128, 128], f32, tag="acc9a", name="acc9a")
            acc9b = psum_pool.tile([128, 128], f32, tag="acc9b", name="acc9b")
            scratch = psumw_pool.tile([128, 512], f32)
            out_sb = out_pool.tile([128, MW], f32)

            # PE pre-warm: dummy matmuls on a zeroed tile keep the PE busy
            # through the HAM activity window while the first chunks stream
            # in, so real matmuls run at 2.4 GHz from the start.
            wz = warm_pool.tile([128, 512], fp8)
            nc.gpsimd.memset(wz[:], 0)
            for _ in range(5):
                nc.tensor.matmul(
                    scratch[:], wz[:, :128], wz[:], start=True, stop=True
                )

            # Input DMAs: rows [off, off+128*t) rearranged so partition p
            # holds its rows contiguously (t*32 bytes per partition line).
            # Each class block is split into two half-DMAs on opposite
            # HWDGE rings (sync/scalar).
            # The SDMA engines round-robin across in-flight transfers at
            # packet granularity, so completions smear late when transfers
            # are large; halving them makes completion sems fire at ~0.3us
            # granularity in class order, which keeps the PE from stalling
            # at class boundaries. Class 0 gets a smaller 16-tile head so
            # the PE can start at the first HBM round-trip.
            z_tiles = {}
            off = 0
            rings = [nc.sync, nc.scalar]
            for j, t in enumerate(tj):
                src = z_dram[off * 128 : (off + t) * 128, :]
                if t >= 20:
                    head = 16 if j == 0 else min(t - 1, (t // 2 + 3) // 4 * 4)
                    za = zin_pool.tile(
                        [128, head * D], fp8, tag=f"z{j}a", name=f"z{j}a"
                    )
                    zb = zin_pool.tile(
                        [128, (t - head) * D], fp8, tag=f"z{j}b", name=f"z{j}b"
                    )
                    rings[j % 2].dma_start(
                        za[:],
                        src[: head * 128, :].rearrange(
                            "(p t) d -> p (t d)", p=128, t=head
                        ),
                    )
                    rings[(j + 1) % 2].dma_start(
                        zb[:],
                        src[head * 128 :, :].rearrange(
                            "(p t) d -> p (t d)", p=128, t=t - head
                        ),
                    )
                    z_tiles[j] = [(za, head), (zb, t - head)]
                else:
                    z_sb = zin_pool.tile(
                        [128, t * D], fp8, tag=f"z{j}", name=f"z{j}"
                    )
                    rings[j % 2].dma_start(
                        z_sb[:],
                        src.rearrange("(p t) d -> p (t d)", p=128, t=t),
                    )
                    z_tiles[j] = [(z_sb, t)]
                off += t

            def quads_of(j):
                for z_sb, tpart in z_tiles[j]:
                    for qq in range(tpart // 4):
                        yield z_sb[:, qq * 128 : (qq + 1) * 128], 128
                    rem = tpart % 4
                    if rem:
                        yield z_sb[:, (tpart // 4) * 128 :], rem * 32

            def evacuate(col, w, acc, ring):
                nc.vector.tensor_copy(out_sb[:, col : col + w], acc[:])
                ring.dma_start(
                    out_dram[:, col : col + w], out_sb[:, col : col + w]
                )

            for g, (lo, hi) in enumerate(groups):
                acc = accs[g]
                for j in range(lo, hi):
                    nmm = tj[j] // 4 + (1 if tj[j] % 4 else 0)
                    for q, (sl, w) in enumerate(quads_of(j)):
                        nc.tensor.matmul(
                            acc[:w, (j - lo) * 128 : (j - lo) * 128 + w],
                            sl,
                            sl,
                            start=(q == 0),
                            stop=(q == nmm - 1),
                        )
                # evacuate this group while later groups keep accumulating
                evacuate(lo * 128, (hi - lo) * 128, acc, rings[g % 2])

            # class 9: peel the last 4 quads into their own PSUM region so
            # the bulk's copy+DMA overlaps them and only a short chain
            # (copy of one region + one small DMA) trails the last matmul
            j9 = C - 1
            nqa = (nq9 - 4) if split9 else nq9
            for q, (sl, w) in enumerate(quads_of(j9)):
                if q < nqa:
                    acc, qq, n = acc9a, q, nqa
                else:
                    acc, qq, n = acc9b, q - nqa, nq9 - nqa
                nc.tensor.matmul(
                    acc[:w, :w], sl, sl, start=(qq == 0), stop=(qq == n - 1)
                )
                if q == nqa - 1 and split9:
                    evacuate(9 * 128, 128, acc9a, rings[1])
            if split9:
                evacuate(10 * 128, 128, acc9b, rings[0])
            else:
                evacuate(9 * 128, 128, acc9a, rings[0])

    nc.compile()
    return nc


def kernel(Z: np.ndarray, labels: np.ndarray) -> np.ndarray:
    Z = np.asarray(Z, dtype=np.float32)
    labels = np.asarray(labels, dtype=np.int32)
    n = Z.shape[0]

    counts = np.bincount(labels, minlength=C)
    # identical per-core capacity per class: ceil(count/8) rounded up to 4 tiles
    tj = []
    for c in counts:
        per_core = -(-int(c) // NCORES)
        tj.append(max(1, -(-per_core // 128)))
    tj = tuple(tj)

    key = tj
    if key not in _cache:
        _cache[key] = _build_program(tj)
    nc = _cache[key]

    ROWS = sum(tj) * 128
    order = np.argsort(labels, kind="stable")
    Zb = Z.astype(ml_dtypes.float8_e4m3)
    bounds = np.concatenate([[0], np.cumsum(counts)])

    in_maps = []
    for k in range(NCORES):
        zp = np.zeros([ROWS, D], ml_dtypes.float8_e4m3)
        off = 0
        for j in range(C):
            cj = int(counts[j])
            s = k * cj // NCORES
            e = (k + 1) * cj // NCORES
            if e > s:
                zp[off : off + (e - s)] = Zb[order[bounds[j] + s : bounds[j] + e]]
            off += tj[j] * 128
        in_maps.append({"Z": zp})

    res = run_bass_kernel_spmd(nc, in_maps, core_ids=list(range(NCORES)))
    _cache["last_results"] = res

    nreg = C + 1 if (tj[C - 1] // 4 + (1 if tj[C - 1] % 4 else 0)) >= 6 else C
    gj = np.zeros([C, D, D], np.float64)
    for r in res.results:
        g = np.asarray(r["grams"], dtype=np.float64)
        for reg in range(nreg):
            j = min(reg, C - 1)
            blk = g[:, reg * 128 : (reg + 1) * 128]
            for a in range(4):
                gj[j] += blk[a * 32 : (a + 1) * 32, a * 32 : (a + 1) * 32]

    g_all = gj.sum(axis=0)
    tr_pi = counts.astype(np.float64)

    nf, df = float(n), float(D)
    eye = np.eye(D)
    loss_r = 0.5 * np.linalg.slogdet(eye + (df / (nf * EPS)) * g_all)[1]
    loss_rc = 0.0
    for j in range(C):
        ld = np.linalg.slogdet(eye + (df / (tr_pi[j] * EPS)) * gj[j])[1]
        loss_rc += (tr_pi[j] / (2.0 * nf)) * ld
    loss_obj = loss_r - loss_rc
    return np.asarray([-loss_obj, loss_r, loss_rc], dtype=np.float32)


# revision 28
# speedup vs baseline: 1.0366x; 1.0366x over previous
"""MCR2 loss kernel for 8 Trainium2 NeuronCores.

Strategy: the host sorts rows by class label (Gram matrices are invariant
to row order), splits each class evenly across the 8 cores, pads each
per-core class block to a multiple of 512 rows (4 tiles of 128), and casts
to fp8 e4m3 (products accumulate exactly in fp32 PSUM; the quantization
error largely cancels between loss_R and loss_Rc, measured 3.1e-3 vs the
f32 reference whose own rounding error is 3.6e-3).  Each core streams its
2.4 MB shard once and accumulates plain per-class Grams on the tensor
engine: for every "quad" of 4 sample tiles, one [128,128] x [128,128]
matmul (lhsT == rhs == the quad) yields the 4 per-tile 32x32 Grams on the
block diagonal of a [128,128] PSUM region; off-diagonal blocks are free
waste.  No masks, no labels, no vector-engine work on the device.

Perf structure (measured 23.9-25 us on a quiet device vs 224.5 us
baseline; the bass framework preamble/epilogue alone measures ~12.8 us
with a trivial kernel, and the PE rhs-stream floor for this dataflow is
150 quads x 56 ns = 8.4 us, which fully hides the 6.7 us fp8 HBM stream):
- each class's input DMA is split in half across the two HWDGE rings
  (sync/scalar) so SDMA packet round-robin cannot smear completion sems
  past the PE's need time; class 0's head is 16 tiles so the PE starts
  at the first HBM round-trip;
- 5 dummy N=512 matmuls on a zeroed tile pre-warm the PE HAM clock gate
  during the first DMA's ~2.7 us latency, so real matmuls run at 2.4 GHz
  (56 ns/quad) without delaying real work when the gate starts warm;
- accumulators are split into per-bank PSUM tiles (classes 0-3 / 4-7 / 8
  / 9-bulk / 9-tail; Tile tracks deps per-tile, so a shared accumulator
  would false-WAR serialize group copies against later matmuls); each
  finished region is copied and DMA'd out (rings alternated) under later
  compute, and class 9's last 4 quads are peeled into their own region so
  only that short copy+DMA chain (~2.6 us, HBM write receipt bound) trails
  the final matmul.

The host sums the diagonal blocks across quads/cores in float64 and
evaluates the 32x32 logdets.
"""

import sys

sys.path.insert(0, "/opt/trn_rl_repo")

import ml_dtypes
import numpy as np

import concourse.bacc as bacc
import concourse.mybir as mybir
import concourse.tile as tile
from concourse.bass_utils import run_bass_kernel_spmd

N, D, C = 600000, 32, 10
EPS = 0.5
NCORES = 8

_cache = {}


def _build_program(tj):
    """tj: per-class tile counts (each a multiple of 4, same on all cores)."""
    TILES = sum(tj)
    ROWS = TILES * 128
    nq9 = tj[C - 1] // 4
    split9 = nq9 >= 6  # peel the last 4 quads of class 9 into their own region
    NREG = C + 1 if split9 else C
    MW = NREG * 128  # one [128,128] f32 region per class (+1 for the peel)

    nc = bacc.Bacc(None)
    fp8 = mybir.dt.float8e4
    f32 = mybir.dt.float32
    z_dram = nc.dram_tensor("Z", [ROWS, D], fp8, kind="ExternalInput")
    out_dram = nc.dram_tensor("grams", [128, MW], f32, kind="ExternalOutput")

    # accumulator groups: classes per PSUM tile (each tile pads to its own
    # bank, so copies of a finished group never serialize against matmuls
    # still accumulating into another group)
    groups = [(0, 4), (4, 8), (8, 9)]

    with tile.TileContext(nc) as tc:
        with (
            tc.tile_pool(name="zin", bufs=1) as zin_pool,
            tc.tile_pool(name="warm", bufs=1) as warm_pool,
            tc.tile_pool(name="outp", bufs=1) as out_pool,
            tc.tile_pool(name="psum", bufs=1, space="PSUM") as psum_pool,
            tc.tile_pool(name="psumw", bufs=1, space="PSUM") as psumw_pool,
        ):
            accs = [
                psum_pool.tile(
                    [128, (hi - lo) * 128], f32, tag=f"acc{g}", name=f"acc{g}"
                )
                for g, (lo, hi) in enumerate(groups)
            ]
            acc9a = psum_pool.tile([128, 128], f32, tag="acc9a", name="acc9a")
            acc9b = psum_pool.tile([128, 128], f32, tag="acc9b", name="acc9b")
            scratch = psumw_pool.tile([128, 512], f32)
            out_sb = out_pool.tile([128, MW], f32)

            # PE pre-warm: dummy matmuls on a zeroed tile keep the PE busy
            # through the HAM activity window while the first chunks stream
            # in, so real matmuls run at 2.4 GHz from the start.
            wz = warm_pool.tile([128, 512], fp8)
            nc.gpsimd.memset(wz[:], 0)
            for _ in range(5):
                nc.tensor.matmul(
                    scratch[:], wz[:, :128], wz[:], start=True, stop=True
                )

            # Input DMAs: rows [off, off+128*t) rearranged so partition p
            # holds its rows contiguously (t*32 bytes per partition line).
            # Each class block is split into two half-DMAs on opposite
            # HWDGE rings (sync/scalar).
            # The SDMA engines round-robin across in-flight transfers at
            # packet granularity, so completions smear late when transfers
            # are large; halving them makes completion sems fire at ~0.3us
            # granularity in class order, which keeps the PE from stalling
            # at class boundaries. Class 0 gets a smaller 16-tile head so
            # the PE can start at the first HBM round-trip.
            z_tiles = {}
            off = 0
            rings = [nc.sync, nc.scalar]
            for j, t in enumerate(tj):
                src = z_dram[off * 128 : (off + t) * 128, :]
                if t >= 20:
                    head = 16 if j == 0 else (t // 2 + 3) // 4 * 4
                    za = zin_pool.tile(
                        [128, head * D], fp8, tag=f"z{j}a", name=f"z{j}a"
                    )
                    zb = zin_pool.tile(
                        [128, (t - head) * D], fp8, tag=f"z{j}b", name=f"z{j}b"
                    )
                    rings[j % 2].dma_start(
                        za[:],
                        src[: head * 128, :].rearrange(
                            "(p t) d -> p (t d)", p=128, t=head
                        ),
                    )
                    rings[(j + 1) % 2].dma_start(
                        zb[:],
                        src[head * 128 :, :].rearrange(
                            "(p t) d -> p (t d)", p=128, t=t - head
                        ),
                    )
                    z_tiles[j] = [(za, head), (zb, t - head)]
                else:
                    z_sb = zin_pool.tile(
                        [128, t * D], fp8, tag=f"z{j}", name=f"z{j}"
                    )
                    rings[j % 2].dma_start(
                        z_sb[:],
                        src.rearrange("(p t) d -> p (t d)", p=128, t=t),
                    )
                    z_tiles[j] = [(z_sb, t)]
                off += t

            def quads_of(j):
                for z_sb, tpart in z_tiles[j]:
                    for qq in range(tpart // 4):
                        yield z_sb[:, qq * 128 : (qq + 1) * 128]

            def evacuate(col, w, acc, ring):
                nc.vector.tensor_copy(out_sb[:, col : col + w], acc[:])
                ring.dma_start(
                    out_dram[:, col : col + w], out_sb[:, col : col + w]
                )

            for g, (lo, hi) in enumerate(groups):
                acc = accs[g]
                for j in range(lo, hi):
                    nq = tj[j] // 4
                    for q, sl in enumerate(quads_of(j)):
                        nc.tensor.matmul(
                            acc[:, (j - lo) * 128 : (j - lo + 1) * 128],
                            sl,
                            sl,
                            start=(q == 0),
                            stop=(q == nq - 1),
                        )
                # evacuate this group while later groups keep accumulating
                evacuate(lo * 128, (hi - lo) * 128, acc, rings[g % 2])

            # class 9: peel the last 4 quads into their own PSUM region so
            # the bulk's copy+DMA overlaps them and only a short chain
            # (copy of one region + one small DMA) trails the last matmul
            j9 = C - 1
            nqa = (nq9 - 4) if split9 else nq9
            for q, sl in enumerate(quads_of(j9)):
                if q < nqa:
                    acc, qq, n = acc9a, q, nqa
                else:
                    acc, qq, n = acc9b, q - nqa, nq9 - nqa
                nc.tensor.matmul(
                    acc[:, :], sl, sl, start=(qq == 0), stop=(qq == n - 1)
                )
                if q == nqa - 1 and split9:
                    evacuate(9 * 128, 128, acc9a, rings[1])
            if split9:
                evacuate(10 * 128, 128, acc9b, rings[0])
            else:
                evacuate(9 * 128, 128, acc9a, rings[0])

    nc.compile()
    return nc


def kernel(Z: np.ndarray, labels: np.ndarray) -> np.ndarray:
    Z = np.asarray(Z, dtype=np.float32)
    labels = np.asarray(labels, dtype=np.int32)
    n = Z.shape[0]

    counts = np.bincount(labels, minlength=C)
    # identical per-core capacity per class: ceil(count/8) rounded up to 4 tiles
    tj = []
    for c in counts:
        per_core = -(-int(c) // NCORES)
        t = -(-per_core // 128)
        tj.append(max(4, (t + 3) // 4 * 4))
    tj = tuple(tj)

    # single-slot program cache: re-executing a previously-built program
    # after a *different* program has run on the cores returns garbage
    # (observed on hardware), so only consecutive same-signature calls
    # reuse the compiled program; a signature change rebuilds fresh.
    if _cache.get("sig") != tj:
        _cache["sig"] = tj
        _cache["nc"] = _build_program(tj)
    nc = _cache["nc"]

    ROWS = sum(tj) * 128
    order = np.argsort(labels, kind="stable")
    Zb = Z.astype(ml_dtypes.float8_e4m3)
    bounds = np.concatenate([[0], np.cumsum(counts)])

    in_maps = []
    for k in range(NCORES):
        zp = np.zeros([ROWS, D], ml_dtypes.float8_e4m3)
        off = 0
        for j in range(C):
            cj = int(counts[j])
            s = k * cj // NCORES
            e = (k + 1) * cj // NCORES
            if e > s:
                zp[off : off + (e - s)] = Zb[order[bounds[j] + s : bounds[j] + e]]
            off += tj[j] * 128
        in_maps.append({"Z": zp})

    res = run_bass_kernel_spmd(nc, in_maps, core_ids=list(range(NCORES)))
    _cache["last_results"] = res

    nreg = C + 1 if (tj[C - 1] // 4) >= 6 else C
    gj = np.zeros([C, D, D], np.float64)
    for r in res.results:
        g = np.asarray(r["grams"], dtype=np.float64)
        for reg in range(nreg):
            j = min(reg, C - 1)
            blk = g[:, reg * 128 : (reg + 1) * 128]
            for a in range(4):
                gj[j] += blk[a * 32 : (a + 1) * 32, a * 32 : (a + 1) * 32]

    g_all = gj.sum(axis=0)
    tr_pi = counts.astype(np.float64)

    nf, df = float(n), float(D)
    eye = np.eye(D)
    loss_r = 0.5 * np.linalg.slogdet(eye + (df / (nf * EPS)) * g_all)[1]
    loss_rc = 0.0
    for j in range(C):
        ld = np.linalg.slogdet(eye + (df / (tr_pi[j] * EPS)) * gj[j])[1]
        loss_rc += (tr_pi[j] / (2.0 * nf)) * ld
    loss_obj = loss_r - loss_rc
    return np.asarray([-loss_obj, loss_r, loss_rc], dtype=np.float32)
